# revision 1
# baseline (speedup 1.0000x reference)
"""Multi-head attention kernel for TRN2, 8 NeuronCores, head-parallel.

Full problem: Q,K,V [B=4, H=8, S=4096, D=64] fp32; out = softmax(QK^T/8) V.
Sharding: 32 (b,h) slices -> 4 per core; no cross-core communication.

Per-core algorithm (heads processed in packed pairs A/B):
  - Prologue per pair, quartered and fully overlapped with compute: DMA K+V_A
    on the sync ring, Q+V_B on the scalar ring (V interleaved after the first
    K/Q quarter); DVE casts each quarter to bf16; Qt/Kt quarter tiles
    [d(A)|d(B) on partitions, s free] built via normal bf16 matmul transposes
    (lhsT=chunk, rhs=identity, ~90ns each, keeps the PE HAM warm). Quarter
    tiles (not one big tile) so the main loop's QK only depends on the
    quarter it reads -> pair 0's main loop starts ~6us in, and pair 1's
    prologue hides inside pair 0's main loop (loads hoisted, transposes
    embedded at qb 4..7).
  - Main loop, one k-chunk (128) per step, per-head psum tiles:
      scoresT[k, q] <- two row-tiled matmuls (A rows 0:64, B rows 64:128)
      issued back-to-back so they run concurrently (~259ns/pair).
      exp with a diagonal engine split to break the single-engine exp wall
      and keep the QK->exp->PV latency chain short (FD=512 per op):
        * (kc + head) even: ScalarE ACTIVATE exact exp (scale=1/8 folded)
        * (kc + head) odd:  VectorE tensor_scalar Schraudolph exp:
          i16 = round(A*s + B) bitcast to bf16 approximates exp(s/8) with
          ~2% element error, zero-mean so softmax renormalization cancels
          the bias; each head gets 50% exact / 50% approx -> ~1e-2 rel err.
      PV: stat = [V_chunk | ones] (65 cols) so the softmax denominator
      accumulates free as row 64 of outT; accumulate over 32 chunks in
      psum [65, 512] per head.
  - Epilogue per (qb, head): outT psum -> sbuf bf16 (both copies on ScalarE),
    transpose back to [q, 65] via 4 normal bf16 matmuls vs identity (fp32
    matmuls are 2-pass with unhideable LDWEIGHTS - avoid), fast reciprocal
    of col 64, scale cols 0:64 (broadcast tensor_tensor), DMA out (sync).

PSUM budget (8 banks): per-head scores [128,512] x4 bufs = 4, PV-out A/B = 2,
transpose scratch = 1, epilogue scratch = 1 (head B's epilogue borrows a
scores buf).
"""

import numpy as np

from concourse import bacc, mybir, tile
from concourse.bass_utils import run_bass_kernel_spmd
from concourse.masks import make_identity

P = 128          # partitions
S = 4096         # sequence length
D = 64           # head dim
NH = 4           # heads per core
NC = S // P      # 32 k-chunks of 128
QB = 512         # q block (psum bank free size in fp32)
NQ = S // QB     # 8 q blocks
NQTR = 4         # DMA quarters
CPQ = NC // NQTR # chunks per quarter
SQ = S // NQTR   # seq elems per quarter
FP32 = mybir.dt.float32
BF16 = mybir.dt.bfloat16
I16 = mybir.dt.int16

N_CORES = 8
SCALE = 1.0 / np.sqrt(np.float32(D))  # 0.125

# Schraudolph exp-as-bf16-bits constants (see module docstring).
# i16 = round(EXP_A * s + EXP_B); bits -> bf16 ~= exp(s * SCALE).
# EXP_A = 128 * log2(e) * SCALE; EXP_B = 128*127 + 128*c0 with c0 chosen so
# the piecewise-linear relative error is zero-mean over f ~ U[0,1).
EXP_A = float(128 * np.log2(np.e) * SCALE)
EXP_B = 16248.7807254998


def build():
    nc = bacc.Bacc("TRN2", target_bir_lowering=False)
    q_d = nc.dram_tensor("Q", (NH, S, D), FP32, kind="ExternalInput")
    k_d = nc.dram_tensor("K", (NH, S, D), FP32, kind="ExternalInput")
    v_d = nc.dram_tensor("V", (NH, S, D), FP32, kind="ExternalInput")
    o_d = nc.dram_tensor("out", (NH, S, D), FP32, kind="ExternalOutput")

    with tile.TileContext(nc) as tc:
        with (
            tc.tile_pool(name="const", bufs=1) as const_pool,
            tc.tile_pool(name="stage", bufs=4) as stage_pool,
            tc.tile_pool(name="stgb", bufs=4) as stgb_pool,
            tc.tile_pool(name="qt", bufs=2) as qt_pool,
            tc.tile_pool(name="kt", bufs=2) as kt_pool,
            tc.tile_pool(name="vsb", bufs=2) as vsb_pool,
            tc.tile_pool(name="pt", bufs=4) as pt_pool,
            tc.tile_pool(name="osb", bufs=2) as osb_pool,
            tc.tile_pool(name="fin", bufs=3) as fin_pool,
            tc.tile_pool(name="recip", bufs=3) as recip_pool,
            tc.tile_pool(name="sc", bufs=4, space="PSUM") as sc_pool,
            tc.tile_pool(name="pso_a", bufs=1, space="PSUM") as pso_a_pool,
            tc.tile_pool(name="pso_b", bufs=1, space="PSUM") as pso_b_pool,
            tc.tile_pool(name="ps_tr", bufs=1, space="PSUM") as ps_tr_pool,
            tc.tile_pool(name="ps_ep", bufs=1, space="PSUM") as ps_ep_pool,
        ):
            ident = const_pool.tile([P, P], BF16)
            make_identity(nc, ident)

            # preload the exp table-set (~2.7us) before any data arrives
            tl_src = const_pool.tile([P, 1], FP32)
            nc.vector.memset(tl_src, 0.0)
            tl_dst = const_pool.tile([P, 1], FP32)
            nc.scalar.activation(
                tl_dst, tl_src, mybir.ActivationFunctionType.Exp, scale=1.0
            )

            def load_quarter(pair, st, g, q_eng, kq_rearr):
                lo = g * CPQ
                kf = stage_pool.tile(
                    [P, CPQ, 2, D], FP32, tag="kf", name=f"kf_{pair}_{g}"
                )
                qf = stage_pool.tile(
                    [P, CPQ, 2, D], FP32, tag="qf", name=f"qf_{pair}_{g}"
                )
                vf = stage_pool.tile(
                    [P, CPQ, 2, D], FP32, tag="vf", name=f"vf_{pair}_{g}"
                )
                for h_i in range(2):
                    kr, qr, vr = kq_rearr[h_i]
                    nc.sync.dma_start(
                        out=kf[:, :, h_i, :], in_=kr[:, lo : lo + CPQ, :]
                    )
                    q_eng.dma_start(
                        out=qf[:, :, h_i, :], in_=qr[:, lo : lo + CPQ, :]
                    )
                nc.sync.dma_start(
                    out=vf[:, :, 0, :], in_=kq_rearr[0][2][:, lo : lo + CPQ, :]
                )
                q_eng.dma_start(
                    out=vf[:, :, 1, :], in_=kq_rearr[1][2][:, lo : lo + CPQ, :]
                )
                st["kf"].append(kf)
                st["qf"].append(qf)
                st["vf"].append(vf)

            def load_pair(pair):
                """Issue all DMA loads for a pair; allocate its big tiles.

                K + V_A ride the sync ring, Q + V_B the scalar ring, with the
                first V quarter right after the first K/Q quarter so the
                first PV chunk's data lands early.
                """
                ha, hb = 2 * pair, 2 * pair + 1
                st = {
                    "qt": [
                        qt_pool.tile([P, SQ], BF16, name=f"qt_{pair}_{g}", tag=f"qt{g}")
                        for g in range(NQTR)
                    ],
                    "kt": [
                        kt_pool.tile([P, SQ], BF16, name=f"kt_{pair}_{g}", tag=f"kt{g}")
                        for g in range(NQTR)
                    ],
                    "vsb": [
                        vsb_pool.tile(
                            [P, 2, CPQ, D + 1], BF16,
                            name=f"vsb_{pair}_{g}", tag=f"vsb{g}",
                        )
                        for g in range(NQTR)
                    ],
                    "kf": [], "qf": [], "vf": [],
                    "heads": (ha, hb),
                }
                for g in range(NQTR):
                    nc.gpsimd.memset(st["vsb"][g][:, :, :, D : D + 1], 1.0)
                kq_rearr = [
                    (k_d[h].rearrange("(c p) d -> p c d", p=P),
                     q_d[h].rearrange("(c p) d -> p c d", p=P),
                     v_d[h].rearrange("(c p) d -> p c d", p=P))
                    for h in (ha, hb)
                ]
                # pair 0: loads woven quarter-by-quarter with prologue
                # compute (issued by the caller); Q/V_B on the idle-at-start
                # scalar queue. Later pairs: everything up-front on the sync
                # queue so the scalar engine is never taxed with DMA issue
                # during the main loop (pair p+1's 6MB fit easily within
                # pair p's ~250us main loop).
                st["q_eng"] = nc.scalar if pair == 0 else nc.sync
                st["kq_rearr"] = kq_rearr
                if pair != 0:
                    for g in range(NQTR):
                        load_quarter(pair, st, g, st["q_eng"], kq_rearr)
                return st

            def prologue_quarter(st, g, fast):
                """bf16 casts + transposes + vsb copy for quarter g.

                fast=True (pair 0 critical path): alternate two psum scratch
                tags for 2-deep pipelining. fast=False (background while the
                previous pair computes): single tag, trickles into idle slots.
                """
                kf, qf, vf = st["kf"][g], st["qf"][g], st["vf"][g]
                kb = stgb_pool.tile(
                    [P, CPQ, 2, D], BF16, tag="kb", name=f"kb_{id(st)}_{g}"
                )
                qb2 = stgb_pool.tile(
                    [P, CPQ, 2, D], BF16, tag="qb", name=f"qb_{id(st)}_{g}"
                )
                half = CPQ // 2
                for hh in range(2):
                    s = slice(hh * half, (hh + 1) * half)
                    nc.vector.tensor_copy(kb[:, s, :, :], kf[:, s, :, :])
                    nc.vector.tensor_copy(qb2[:, s, :, :], qf[:, s, :, :])
                for c in range(CPQ):
                    for t_i, (src, dst) in enumerate(
                        ((kb, st["kt"][g]), (qb2, st["qt"][g]))
                    ):
                        if fast:
                            tag, pool = (
                                ("ps_tr", ps_tr_pool)
                                if (2 * c + t_i) % 2 == 0
                                else ("ps_ep", ps_ep_pool)
                            )
                        else:
                            tag, pool = "ps_tr", ps_tr_pool
                        ps_t = pool.tile([P, P], FP32, tag=tag)
                        nc.tensor.matmul(
                            ps_t,
                            lhsT=src[:, c, :, :].rearrange("p a b -> p (a b)"),
                            rhs=ident,
                            start=True,
                            stop=True,
                        )
                        col = dst[:, c * P : (c + 1) * P]
                        if t_i == 0:
                            nc.scalar.copy(col, ps_t)
                        else:
                            nc.vector.tensor_copy(col, ps_t)
                half = CPQ // 2
                for h_i in range(2):
                    for hh in range(2):
                        s = slice(hh * half, (hh + 1) * half)
                        nc.vector.tensor_copy(
                            st["vsb"][g][:, h_i, s, 0:D], vf[:, s, h_i, :]
                        )

            def main_pair(st, next_st):
                ha, hb = st["heads"]
                for qb in range(NQ):
                    q0 = (qb % 2) * QB
                    qt_q = st["qt"][qb // 2]
                    out_ta = pso_a_pool.tile([D + 1, QB], FP32)
                    out_tb = pso_b_pool.tile([D + 1, QB], FP32)
                    for kc in range(NC):
                        g, cq = kc // CPQ, kc % CPQ
                        kt_q = st["kt"][g]
                        sca = sc_pool.tile([P, QB], FP32, tag="sc")
                        scb = sc_pool.tile([P, QB], FP32, tag="sc")
                        nc.tensor.matmul(
                            sca,
                            lhsT=kt_q[0:64, cq * P : (cq + 1) * P],
                            rhs=qt_q[0:64, q0 : q0 + QB],
                            start=True,
                            stop=True,
                        )
                        nc.tensor.matmul(
                            scb,
                            lhsT=kt_q[64:128, cq * P : (cq + 1) * P],
                            rhs=qt_q[64:128, q0 : q0 + QB],
                            start=True,
                            stop=True,
                        )
                        pts = []
                        for h_i, sc in enumerate((sca, scb)):
                            if (kc + h_i) % 2 == 0:
                                pt = pt_pool.tile([P, QB], BF16, tag=f"pt{h_i}")
                                nc.scalar.activation(
                                    pt,
                                    sc,
                                    mybir.ActivationFunctionType.Exp,
                                    scale=SCALE,
                                )
                                pts.append(pt)
                            else:
                                pt_i = pt_pool.tile([P, QB], I16, tag=f"pt{h_i}")
                                nc.vector.tensor_scalar(
                                    out=pt_i,
                                    in0=sc,
                                    scalar1=EXP_A,
                                    scalar2=EXP_B,
                                    op0=mybir.AluOpType.mult,
                                    op1=mybir.AluOpType.add,
                                )
                                pts.append(pt_i.bitcast(BF16))
                        first = kc == 0
                        last = kc == NC - 1
                        nc.tensor.matmul(
                            out_ta,
                            lhsT=st["vsb"][g][:, 0, cq, :],
                            rhs=pts[0],
                            start=first,
                            stop=last,
                        )
                        nc.tensor.matmul(
                            out_tb,
                            lhsT=st["vsb"][g][:, 1, cq, :],
                            rhs=pts[1],
                            start=first,
                            stop=last,
                        )

                    # ---- epilogue: transpose back, normalize, store ----
                    qd = qb * QB
                    for h_i, (h, out_t) in enumerate(((ha, out_ta), (hb, out_tb))):
                        osb = osb_pool.tile([D + 1, QB], BF16)
                        nc.scalar.copy(osb, out_t)
                        if h_i == 0:
                            ps4 = ps_ep_pool.tile(
                                [P, QB // P, D + 1], FP32, tag="ps_ep"
                            )
                        else:
                            ps4 = sc_pool.tile([P, QB // P, D + 1], FP32, tag="sc")
                        for j in range(QB // P):
                            nc.tensor.matmul(
                                ps4[:, j, :],
                                lhsT=osb[:, j * P : (j + 1) * P],
                                rhs=ident[0 : D + 1, 0 : D + 1],
                                start=True,
                                stop=True,
                            )
                        rec = recip_pool.tile([P, QB // P, 1], FP32)
                        nc.vector.reciprocal_approx_fast(rec, ps4[:, :, D : D + 1])
                        fin = fin_pool.tile([P, QB // P, D], FP32)
                        nc.vector.tensor_tensor(
                            fin,
                            ps4[:, :, 0:D],
                            rec.broadcast_to((P, QB // P, D)),
                            mybir.AluOpType.mult,
                        )
                        nc.sync.dma_start(
                            out=o_d[h, qd : qd + QB, :].rearrange(
                                "(j p) d -> p j d", p=P
                            ),
                            in_=fin,
                        )

                    # background prologue for the next pair during qb 4..7
                    if next_st is not None and qb >= NQ - NQTR:
                        prologue_quarter(next_st, qb - (NQ - NQTR), fast=False)

            st0 = load_pair(0)
            for g in range(NQTR):
                load_quarter(0, st0, g, st0["q_eng"], st0["kq_rearr"])
                prologue_quarter(st0, g, fast=True)
            st1 = load_pair(1)
            main_pair(st0, st1)
            main_pair(st1, None)

    nc.compile()
    return nc


_NC_CACHE = None


def _get_nc():
    global _NC_CACHE
    if _NC_CACHE is None:
        _NC_CACHE = build()
    return _NC_CACHE


def kernel(Q, K, V):
    Q = np.ascontiguousarray(np.asarray(Q, dtype=np.float32))
    K = np.ascontiguousarray(np.asarray(K, dtype=np.float32))
    V = np.ascontiguousarray(np.asarray(V, dtype=np.float32))
    B, H = Q.shape[0], Q.shape[1]
    qr = Q.reshape(B * H, S, D)
    kr = K.reshape(B * H, S, D)
    vr = V.reshape(B * H, S, D)
    in_maps = [
        {
            "Q": qr[i * NH : (i + 1) * NH],
            "K": kr[i * NH : (i + 1) * NH],
            "V": vr[i * NH : (i + 1) * NH],
        }
        for i in range(N_CORES)
    ]
    res = run_bass_kernel_spmd(_get_nc(), in_maps, core_ids=list(range(N_CORES)))
    out = np.concatenate([res.results[i]["out"] for i in range(N_CORES)], axis=0)
    return out.reshape(B, H, S, D)



# revision 10
# speedup vs baseline: 1.0235x; 1.0235x over previous
"""Multi-head attention kernel for TRN2, 8 NeuronCores, head-parallel. v2.

Full problem: Q,K,V [B=4, H=8, S=4096, D=64] fp32; out = softmax(QK^T/8) V.
Sharding: 32 (b,h) slices -> 4 per core; no cross-core communication.

Per-core algorithm (heads processed in packed pairs A/B):
  - Prologue per pair, all DMA (no compute-engine work): per (tensor, head,
    quarter): (1) gpsimd SWDGE DMA casts HBM fp32 -> SBUF bf16 in flight;
    (2) sync DMA bounces the bf16 rows back to a DRAM staging tensor;
    (3) sync 16-bit xbar-transpose DMA reads the staged [1024, 64] rows and
    writes [64, 1024] directly into qt/kt [128, S] bf16 (head A partitions
    0:64, head B 64:128).  (2)->(3) ordering is guaranteed by same-queue
    FIFO; (1)->(2) by Tile tracking of the SBUF stage tile.  V loads are
    gpsimd cast-DMAs straight into vsb [128, 2, NC, 65] bf16 (ones column at
    col 64: softmax denominator accumulates free as row 64 of PV output).
  - Main loop, one k-chunk (128) per step, per-qb(512) psum accumulators:
      scoresT[k, 2, q] <- one 2-bank psum supertile per chunk holding BOTH
      heads ([:,0,:] head A via kt rows 0:64, [:,1,:] head B via rows 64:128);
      the two QK matmuls are row-tile concurrent (disjoint row groups).
      exp: ONE FD=1024 op per chunk covering both heads, alternating engines
      by chunk parity to halve the per-op overhead (352cyc ScalarE/120cyc DVE):
        * kc even: ScalarE ACTIVATE exact exp (scale=1/8 folded)
        * kc odd:  VectorE tensor_scalar Schraudolph exp (i16 bitcast bf16,
          ~2% element error, zero-mean so softmax renormalization cancels)
      PV: lhsT = [V_chunk | ones] (65 cols); 2 matmuls (heads A/B) accumulate
      into [65, 512] psum over 32 chunks.
  - Epilogue per (qb, head): outT psum -> sbuf bf16 (head A copy on ScalarE,
    head B on VectorE), transpose back via 4 bf16 matmuls vs identity into
    scratch placed in the just-freed pso bank, fast reciprocal of col 64,
    broadcast multiply, DMA out on the gpsimd queue.

PSUM budget (8 banks): score supertiles [128,1024] x3 bufs = 6 banks,
PV-out A/B = 2 banks; epilogue transpose scratch reuses the pso banks.

Engine budget per chunk (~650ns pace): PE 1536cyc/2.4GHz = 640ns
(QK pair 512 + PV 2x512); ScalarE ~574ns (exp every other chunk) + epilogue
copies; VectorE ~596ns + casts + epilogue recip/mul; GpSimd: DMA issue only.
"""

import numpy as np

from concourse import bacc, mybir, tile
from concourse.bass_utils import run_bass_kernel_spmd
from concourse.masks import make_identity

P = 128          # partitions
S = 4096         # sequence length
D = 64           # head dim
NH = 4           # heads per core
NC = S // P      # 32 k-chunks of 128
QB = 512         # q block (psum bank free size in fp32)
NQ = S // QB     # 8 q blocks
NQTR = 4         # DMA quarters
CPQ = NC // NQTR # chunks per quarter (8)
SQ = S // NQTR   # seq elems per quarter (1024)
FP32 = mybir.dt.float32
BF16 = mybir.dt.bfloat16
I16 = mybir.dt.int16

N_CORES = 8
SCALE = 1.0 / np.sqrt(np.float32(D))  # 0.125

# Schraudolph exp-as-bf16-bits constants (see module docstring).
# i16 = round(EXP_A * s + EXP_B); bits -> bf16 ~= exp(s * SCALE).
EXP_A = float(128 * np.log2(np.e) * SCALE)
EXP_B = 16248.7807254998


def build():
    nc = bacc.Bacc("TRN2", target_bir_lowering=False)
    q_d = nc.dram_tensor("Q", (NH, S, D), FP32, kind="ExternalInput")
    k_d = nc.dram_tensor("K", (NH, S, D), FP32, kind="ExternalInput")
    v_d = nc.dram_tensor("V", (NH, S, D), FP32, kind="ExternalInput")
    o_d = nc.dram_tensor("out", (NH, S, D), FP32, kind="ExternalOutput")
    # DRAM bounce buffers for the bf16 transpose: rows interleave the two
    # heads of a pair ([s, 2, D] -> transpose input [s, 128]).
    qstg_d = nc.dram_tensor("qstg", (NH // 2, S, 2, D), BF16, kind="Internal")
    kstg_d = nc.dram_tensor("kstg", (NH // 2, S, 2, D), BF16, kind="Internal")

    with tile.TileContext(nc) as tc:
        with (
            tc.tile_pool(name="const", bufs=1) as const_pool,
            tc.tile_pool(name="stg", bufs=3) as stg_pool,
            tc.tile_pool(name="qt", bufs=2) as qt_pool,
            tc.tile_pool(name="kt", bufs=2) as kt_pool,
            tc.tile_pool(name="vsb", bufs=2) as vsb_pool,
            tc.tile_pool(name="pt", bufs=3) as pt_pool,
            tc.tile_pool(name="osb", bufs=3) as osb_pool,
            tc.tile_pool(name="fin", bufs=4) as fin_pool,
            tc.tile_pool(name="recip", bufs=3) as recip_pool,
            tc.tile_pool(name="sc", bufs=3, space="PSUM") as sc_pool,
            tc.tile_pool(name="pso_a", bufs=1, space="PSUM") as pso_a_pool,
            tc.tile_pool(name="pso_b", bufs=1, space="PSUM") as pso_b_pool,
        ):
            ident = const_pool.tile([P, P], BF16)
            make_identity(nc, ident)

            # preload the exp table-set (~2.7us) before any data arrives
            tl_src = const_pool.tile([P, 1], FP32)
            nc.vector.memset(tl_src, 0.0)
            tl_dst = const_pool.tile([P, 1], FP32)
            nc.scalar.activation(
                tl_dst, tl_src, mybir.ActivationFunctionType.Exp, scale=1.0
            )

            def load_pair(pair):
                """Issue all loads for a pair (quartered), all on DMA engines.

                Per (tensor, head, quarter): gpsimd cast-DMA HBM fp32 ->
                SBUF bf16 rows; sync DMA SBUF -> DRAM bf16 staging; sync
                16-bit transpose DMA DRAM [SQ, D] -> qt/kt [64, SQ] slice.
                V: gpsimd cast-DMA straight into vsb.
                """
                ha, hb = 2 * pair, 2 * pair + 1
                qt = qt_pool.tile([P, S], BF16, name=f"qt_{pair}", tag="qt")
                kt = kt_pool.tile([P, S], BF16, name=f"kt_{pair}", tag="kt")
                vsb = vsb_pool.tile(
                    [P, 2, NC, D + 1], BF16, name=f"vsb_{pair}", tag="vsb"
                )
                nc.gpsimd.memset(vsb[:, :, :, D : D + 1], 1.0)
                for g in range(NQTR):
                    lo = g * SQ
                    clo = g * CPQ
                    for x_d, xstg_d, xt in (
                        (q_d, qstg_d, qt),
                        (k_d, kstg_d, kt),
                    ):
                        for h_i, h in enumerate((ha, hb)):
                            stg = stg_pool.tile(
                                [P, CPQ, D], BF16, tag="stg"
                            )
                            nc.gpsimd.dma_start(
                                out=stg,
                                in_=x_d[h][lo : lo + SQ, :].rearrange(
                                    "(c p) d -> p c d", p=P
                                ),
                            )
                            nc.sync.dma_start(
                                out=xstg_d[pair, lo : lo + SQ, h_i, :].rearrange(
                                    "(c p) d -> p c d", p=P
                                ),
                                in_=stg,
                            )
                        nc.sync.dma_start(
                            out=xt[:, lo : lo + SQ],
                            in_=xstg_d[pair, lo : lo + SQ].rearrange(
                                "s h d -> s (h d)"
                            ),
                            transpose=True,
                        )
                    for h_i, h in enumerate((ha, hb)):
                        nc.gpsimd.dma_start(
                            out=vsb[:, h_i, clo : clo + CPQ, 0:D],
                            in_=v_d[h].rearrange("(c p) d -> p c d", p=P)[
                                :, clo : clo + CPQ, :
                            ],
                        )
                return {"qt": qt, "kt": kt, "vsb": vsb, "heads": (ha, hb)}

            def main_pair(st):
                qt, kt, vsb = st["qt"], st["kt"], st["vsb"]
                ha, hb = st["heads"]
                for qb in range(NQ):
                    q0 = qb * QB
                    out_ta = pso_a_pool.tile([D + 1, QB], FP32, tag="pso_a")
                    out_tb = pso_b_pool.tile([D + 1, QB], FP32, tag="pso_b")
                    for kc in range(NC):
                        sc = sc_pool.tile([P, 2, QB], FP32, tag="sc")
                        nc.tensor.matmul(
                            sc[:, 0, :],
                            lhsT=kt[0:64, kc * P : (kc + 1) * P],
                            rhs=qt[0:64, q0 : q0 + QB],
                            start=True,
                            stop=True,
                        )
                        nc.tensor.matmul(
                            sc[:, 1, :],
                            lhsT=kt[64:128, kc * P : (kc + 1) * P],
                            rhs=qt[64:128, q0 : q0 + QB],
                            start=True,
                            stop=True,
                        )
                        if kc % 2 == 0:
                            pt = pt_pool.tile([P, 2, QB], BF16, tag="pt")
                            nc.scalar.activation(
                                pt,
                                sc,
                                mybir.ActivationFunctionType.Exp,
                                scale=SCALE,
                            )
                        else:
                            pt_i = pt_pool.tile([P, 2, QB], I16, tag="pt")
                            nc.vector.tensor_scalar(
                                out=pt_i,
                                in0=sc,
                                scalar1=EXP_A,
                                scalar2=EXP_B,
                                op0=mybir.AluOpType.mult,
                                op1=mybir.AluOpType.add,
                            )
                            pt = pt_i.bitcast(BF16)
                        first = kc == 0
                        last = kc == NC - 1
                        nc.tensor.matmul(
                            out_ta,
                            lhsT=vsb[:, 0, kc, :],
                            rhs=pt[:, 0, :],
                            start=first,
                            stop=last,
                        )
                        nc.tensor.matmul(
                            out_tb,
                            lhsT=vsb[:, 1, kc, :],
                            rhs=pt[:, 1, :],
                            start=first,
                            stop=last,
                        )

                    # ---- epilogue: transpose back, normalize, store ----
                    qd = qb * QB
                    for h_i, (h, out_t, pool) in enumerate(
                        ((ha, out_ta, pso_a_pool), (hb, out_tb, pso_b_pool))
                    ):
                        osb = osb_pool.tile([D + 1, QB], BF16)
                        nc.scalar.copy(osb, out_t)
                        # transpose scratch reuses the (just freed) pso bank
                        ps4 = pool.tile([P, QB // P, D + 1], FP32, tag="pso_a" if h_i == 0 else "pso_b")
                        for j in range(QB // P):
                            nc.tensor.matmul(
                                ps4[:, j, :],
                                lhsT=osb[:, j * P : (j + 1) * P],
                                rhs=ident[0 : D + 1, 0 : D + 1],
                                start=True,
                                stop=True,
                            )
                        rec = recip_pool.tile([P, QB // P, 1], FP32)
                        nc.vector.reciprocal_approx_fast(rec, ps4[:, :, D : D + 1])
                        fin = fin_pool.tile([P, QB // P, D], FP32)
                        nc.vector.tensor_tensor(
                            fin,
                            ps4[:, :, 0:D],
                            rec.broadcast_to((P, QB // P, D)),
                            mybir.AluOpType.mult,
                        )
                        nc.gpsimd.dma_start(
                            out=o_d[h, qd : qd + QB, :].rearrange(
                                "(j p) d -> p j d", p=P
                            ),
                            in_=fin,
                        )

            st0 = load_pair(0)
            st1 = load_pair(1)
            main_pair(st0)
            main_pair(st1)

    nc.compile()
    return nc


_NC_CACHE = None


def _get_nc():
    global _NC_CACHE
    if _NC_CACHE is None:
        _NC_CACHE = build()
    return _NC_CACHE


def kernel(Q, K, V):
    Q = np.ascontiguousarray(np.asarray(Q, dtype=np.float32))
    K = np.ascontiguousarray(np.asarray(K, dtype=np.float32))
    V = np.ascontiguousarray(np.asarray(V, dtype=np.float32))
    B, H = Q.shape[0], Q.shape[1]
    qr = Q.reshape(B * H, S, D)
    kr = K.reshape(B * H, S, D)
    vr = V.reshape(B * H, S, D)
    in_maps = [
        {
            "Q": qr[i * NH : (i + 1) * NH],
            "K": kr[i * NH : (i + 1) * NH],
            "V": vr[i * NH : (i + 1) * NH],
        }
        for i in range(N_CORES)
    ]
    res = run_bass_kernel_spmd(_get_nc(), in_maps, core_ids=list(range(N_CORES)))
    out = np.concatenate([res.results[i]["out"] for i in range(N_CORES)], axis=0)
    return out.reshape(B, H, S, D)


# revision 12
# speedup vs baseline: 1.0439x; 1.0199x over previous
"""Multi-head attention kernel for TRN2, 8 NeuronCores, head-parallel. v2.

Full problem: Q,K,V [B=4, H=8, S=4096, D=64] fp32; out = softmax(QK^T/8) V.
Sharding: 32 (b,h) slices -> 4 per core; no cross-core communication.

Per-core algorithm (heads processed in packed pairs A/B):
  - Prologue per pair, all DMA (no compute-engine work): per (tensor, head,
    quarter): (1) gpsimd SWDGE DMA casts HBM fp32 -> SBUF bf16 in flight;
    (2) sync DMA bounces the bf16 rows back to a DRAM staging tensor;
    (3) sync 16-bit xbar-transpose DMA reads the staged [1024, 64] rows and
    writes [64, 1024] directly into qt/kt [128, S] bf16 (head A partitions
    0:64, head B 64:128).  (2)->(3) ordering is guaranteed by same-queue
    FIFO; (1)->(2) by Tile tracking of the SBUF stage tile.  V loads are
    gpsimd cast-DMAs straight into vsb [128, 2, NC, 65] bf16 (ones column at
    col 64: softmax denominator accumulates free as row 64 of PV output).
  - Main loop, one k-chunk (128) per step, per-qb(512) psum accumulators:
      scoresT[k, 2, q] <- one 2-bank psum supertile per chunk holding BOTH
      heads ([:,0,:] head A via kt rows 0:64, [:,1,:] head B via rows 64:128);
      the two QK matmuls are row-tile concurrent (disjoint row groups).
      exp: ONE FD=1024 op per chunk covering both heads, alternating engines
      by chunk parity to halve the per-op overhead (352cyc ScalarE/120cyc DVE):
        * kc even: ScalarE ACTIVATE exact exp (scale=1/8 folded)
        * kc odd:  VectorE tensor_scalar Schraudolph exp (i16 bitcast bf16,
          ~2% element error, zero-mean so softmax renormalization cancels)
      PV: lhsT = [V_chunk | ones] (65 cols); 2 matmuls (heads A/B) accumulate
      into [65, 512] psum over 32 chunks.
  - Epilogue per (qb, head): outT psum -> sbuf bf16 (head A copy on ScalarE,
    head B on VectorE), transpose back via 4 bf16 matmuls vs identity into
    scratch placed in the just-freed pso bank, fast reciprocal of col 64,
    broadcast multiply, DMA out on the gpsimd queue.

PSUM budget (8 banks): score supertiles [128,1024] x3 bufs = 6 banks,
PV-out A/B = 2 banks; epilogue transpose scratch reuses the pso banks.

Engine budget per chunk (~650ns pace): PE 1536cyc/2.4GHz = 640ns
(QK pair 512 + PV 2x512); ScalarE ~574ns (exp every other chunk) + epilogue
copies; VectorE ~596ns + casts + epilogue recip/mul; GpSimd: DMA issue only.
"""

import numpy as np

from concourse import bacc, mybir, tile
from concourse.bass_utils import run_bass_kernel_spmd
from concourse.masks import make_identity

P = 128          # partitions
S = 4096         # sequence length
D = 64           # head dim
NH = 4           # heads per core
NC = S // P      # 32 k-chunks of 128
QB = 512         # q block (psum bank free size in fp32)
NQ = S // QB     # 8 q blocks
NQTR = 4         # DMA quarters
CPQ = NC // NQTR # chunks per quarter (8)
SQ = S // NQTR   # seq elems per quarter (1024)
FP32 = mybir.dt.float32
BF16 = mybir.dt.bfloat16
I16 = mybir.dt.int16

N_CORES = 8
SCALE = 1.0 / np.sqrt(np.float32(D))  # 0.125

# Schraudolph exp-as-bf16-bits constants (see module docstring).
# i16 = round(EXP_A * s + EXP_B); bits -> bf16 ~= exp(s * SCALE).
EXP_A = float(128 * np.log2(np.e) * SCALE)
EXP_B = 16248.7807254998


def build():
    nc = bacc.Bacc("TRN2", target_bir_lowering=False)
    q_d = nc.dram_tensor("Q", (NH, S, D), FP32, kind="ExternalInput")
    k_d = nc.dram_tensor("K", (NH, S, D), FP32, kind="ExternalInput")
    v_d = nc.dram_tensor("V", (NH, S, D), FP32, kind="ExternalInput")
    o_d = nc.dram_tensor("out", (NH, S, D), FP32, kind="ExternalOutput")
    # DRAM bounce buffers for the bf16 transpose: rows interleave the two
    # heads of a pair ([s, 2, D] -> transpose input [s, 128]).
    qstg_d = nc.dram_tensor("qstg", (NH // 2, S, 2, D), BF16, kind="Internal")
    kstg_d = nc.dram_tensor("kstg", (NH // 2, S, 2, D), BF16, kind="Internal")

    with tile.TileContext(nc) as tc:
        with (
            tc.tile_pool(name="const", bufs=1) as const_pool,
            tc.tile_pool(name="stg", bufs=3) as stg_pool,
            tc.tile_pool(name="qt", bufs=2) as qt_pool,
            tc.tile_pool(name="kt", bufs=2) as kt_pool,
            tc.tile_pool(name="vsb", bufs=2) as vsb_pool,
            tc.tile_pool(name="pt", bufs=3) as pt_pool,
            tc.tile_pool(name="osb", bufs=3) as osb_pool,
            tc.tile_pool(name="fin", bufs=4) as fin_pool,
            tc.tile_pool(name="recip", bufs=3) as recip_pool,
            tc.tile_pool(name="sc", bufs=3, space="PSUM") as sc_pool,
            tc.tile_pool(name="pso_a", bufs=1, space="PSUM") as pso_a_pool,
            tc.tile_pool(name="pso_b", bufs=1, space="PSUM") as pso_b_pool,
        ):
            ident = const_pool.tile([P, P], BF16)
            make_identity(nc, ident)

            # preload the exp table-set (~2.7us) before any data arrives
            tl_src = const_pool.tile([P, 1], FP32)
            nc.vector.memset(tl_src, 0.0)
            tl_dst = const_pool.tile([P, 1], FP32)
            nc.scalar.activation(
                tl_dst, tl_src, mybir.ActivationFunctionType.Exp, scale=1.0
            )

            def load_pair(pair):
                """Issue all loads for a pair (quartered), all on DMA engines.

                Per (tensor, head, quarter): gpsimd cast-DMA HBM fp32 ->
                SBUF bf16 rows; sync DMA SBUF -> DRAM bf16 staging; sync
                16-bit transpose DMA DRAM [SQ, D] -> qt/kt [64, SQ] slice.
                V: gpsimd cast-DMA straight into vsb.
                """
                ha, hb = 2 * pair, 2 * pair + 1
                qt = qt_pool.tile([P, S], BF16, name=f"qt_{pair}", tag="qt")
                kt = kt_pool.tile([P, S], BF16, name=f"kt_{pair}", tag="kt")
                vsb = vsb_pool.tile(
                    [P, 2, NC, D + 1], BF16, name=f"vsb_{pair}", tag="vsb"
                )
                nc.gpsimd.memset(vsb[:, :, :, D : D + 1], 1.0)
                for g in range(NQTR):
                    lo = g * SQ
                    clo = g * CPQ
                    for x_d, xstg_d, xt in (
                        (q_d, qstg_d, qt),
                        (k_d, kstg_d, kt),
                    ):
                        for h_i, h in enumerate((ha, hb)):
                            stg = stg_pool.tile(
                                [P, CPQ, D], BF16, tag="stg"
                            )
                            nc.gpsimd.dma_start(
                                out=stg,
                                in_=x_d[h][lo : lo + SQ, :].rearrange(
                                    "(c p) d -> p c d", p=P
                                ),
                            )
                            nc.sync.dma_start(
                                out=xstg_d[pair, lo : lo + SQ, h_i, :].rearrange(
                                    "(c p) d -> p c d", p=P
                                ),
                                in_=stg,
                            )
                        nc.sync.dma_start(
                            out=xt[:, lo : lo + SQ],
                            in_=xstg_d[pair, lo : lo + SQ].rearrange(
                                "s h d -> s (h d)"
                            ),
                            transpose=True,
                        )
                    for h_i, h in enumerate((ha, hb)):
                        nc.gpsimd.dma_start(
                            out=vsb[:, h_i, clo : clo + CPQ, 0:D],
                            in_=v_d[h].rearrange("(c p) d -> p c d", p=P)[
                                :, clo : clo + CPQ, :
                            ],
                        )
                return {"qt": qt, "kt": kt, "vsb": vsb, "heads": (ha, hb)}

            LAG = 2  # PV trails QK by LAG chunks (software pipelining: the
            # in-order PE queue must not park on a PV whose exp isn't done)

            def main_pair(st):
                qt, kt, vsb = st["qt"], st["kt"], st["vsb"]
                ha, hb = st["heads"]
                for qb in range(NQ):
                    q0 = qb * QB
                    out_ta = pso_a_pool.tile([D + 1, QB], FP32, tag="pso_a")
                    out_tb = pso_b_pool.tile([D + 1, QB], FP32, tag="pso_b")
                    pend = []

                    def issue_pv(pt, kc):
                        first = kc == 0
                        last = kc == NC - 1
                        nc.tensor.matmul(
                            out_ta,
                            lhsT=vsb[:, 0, kc, :],
                            rhs=pt[:, 0, :],
                            start=first,
                            stop=last,
                        )
                        nc.tensor.matmul(
                            out_tb,
                            lhsT=vsb[:, 1, kc, :],
                            rhs=pt[:, 1, :],
                            start=first,
                            stop=last,
                        )

                    for kc in range(NC):
                        sc = sc_pool.tile([P, 2, QB], FP32, tag="sc")
                        nc.tensor.matmul(
                            sc[:, 0, :],
                            lhsT=kt[0:64, kc * P : (kc + 1) * P],
                            rhs=qt[0:64, q0 : q0 + QB],
                            start=True,
                            stop=True,
                        )
                        nc.tensor.matmul(
                            sc[:, 1, :],
                            lhsT=kt[64:128, kc * P : (kc + 1) * P],
                            rhs=qt[64:128, q0 : q0 + QB],
                            start=True,
                            stop=True,
                        )
                        if kc % 2 == 0:
                            pt = pt_pool.tile([P, 2, QB], BF16, tag="pt")
                            nc.scalar.activation(
                                pt,
                                sc,
                                mybir.ActivationFunctionType.Exp,
                                scale=SCALE,
                            )
                        else:
                            pt_i = pt_pool.tile([P, 2, QB], I16, tag="pt")
                            nc.vector.tensor_scalar(
                                out=pt_i,
                                in0=sc,
                                scalar1=EXP_A,
                                scalar2=EXP_B,
                                op0=mybir.AluOpType.mult,
                                op1=mybir.AluOpType.add,
                            )
                            pt = pt_i.bitcast(BF16)
                        pend.append((pt, kc))
                        if len(pend) > LAG:
                            issue_pv(*pend.pop(0))
                    for args in pend:
                        issue_pv(*args)

                    # ---- epilogue: transpose back, normalize, store ----
                    qd = qb * QB
                    for h_i, (h, out_t, pool) in enumerate(
                        ((ha, out_ta, pso_a_pool), (hb, out_tb, pso_b_pool))
                    ):
                        osb = osb_pool.tile([D + 1, QB], BF16)
                        if h_i == 0:
                            nc.scalar.copy(osb, out_t)
                        else:
                            nc.vector.tensor_copy(osb, out_t)
                        # transpose scratch reuses the (just freed) pso bank
                        ps4 = pool.tile([P, QB // P, D + 1], FP32, tag="pso_a" if h_i == 0 else "pso_b")
                        for j in range(QB // P):
                            nc.tensor.matmul(
                                ps4[:, j, :],
                                lhsT=osb[:, j * P : (j + 1) * P],
                                rhs=ident[0 : D + 1, 0 : D + 1],
                                start=True,
                                stop=True,
                            )
                        rec = recip_pool.tile([P, QB // P, 1], FP32)
                        nc.vector.reciprocal_approx_fast(rec, ps4[:, :, D : D + 1])
                        fin = fin_pool.tile([P, QB // P, D], FP32)
                        nc.vector.tensor_tensor(
                            fin,
                            ps4[:, :, 0:D],
                            rec.broadcast_to((P, QB // P, D)),
                            mybir.AluOpType.mult,
                        )
                        nc.gpsimd.dma_start(
                            out=o_d[h, qd : qd + QB, :].rearrange(
                                "(j p) d -> p j d", p=P
                            ),
                            in_=fin,
                        )

            st0 = load_pair(0)
            st1 = load_pair(1)
            main_pair(st0)
            main_pair(st1)

    nc.compile()
    return nc


_NC_CACHE = None


def _get_nc():
    global _NC_CACHE
    if _NC_CACHE is None:
        _NC_CACHE = build()
    return _NC_CACHE


def kernel(Q, K, V):
    Q = np.ascontiguousarray(np.asarray(Q, dtype=np.float32))
    K = np.ascontiguousarray(np.asarray(K, dtype=np.float32))
    V = np.ascontiguousarray(np.asarray(V, dtype=np.float32))
    B, H = Q.shape[0], Q.shape[1]
    qr = Q.reshape(B * H, S, D)
    kr = K.reshape(B * H, S, D)
    vr = V.reshape(B * H, S, D)
    in_maps = [
        {
            "Q": qr[i * NH : (i + 1) * NH],
            "K": kr[i * NH : (i + 1) * NH],
            "V": vr[i * NH : (i + 1) * NH],
        }
        for i in range(N_CORES)
    ]
    res = run_bass_kernel_spmd(_get_nc(), in_maps, core_ids=list(range(N_CORES)))
    out = np.concatenate([res.results[i]["out"] for i in range(N_CORES)], axis=0)
    return out.reshape(B, H, S, D)


# revision 15
# speedup vs baseline: 1.0800x; 1.0346x over previous
"""Multi-head attention kernel for TRN2, 8 NeuronCores, head-parallel. v2.

Full problem: Q,K,V [B=4, H=8, S=4096, D=64] fp32; out = softmax(QK^T/8) V.
Sharding: 32 (b,h) slices -> 4 per core; no cross-core communication.

Per-core algorithm (heads processed in packed pairs A/B):
  - Prologue per pair, all DMA (no compute-engine work): per (tensor, head,
    quarter): (1) gpsimd SWDGE DMA casts HBM fp32 -> SBUF bf16 in flight;
    (2) sync DMA bounces the bf16 rows back to a DRAM staging tensor;
    (3) sync 16-bit xbar-transpose DMA reads the staged [1024, 64] rows and
    writes [64, 1024] directly into qt/kt [128, S] bf16 (head A partitions
    0:64, head B 64:128).  (2)->(3) ordering is guaranteed by same-queue
    FIFO; (1)->(2) by Tile tracking of the SBUF stage tile.  V loads are
    gpsimd cast-DMAs straight into vsb [128, 2, NC, 65] bf16 (ones column at
    col 64: softmax denominator accumulates free as row 64 of PV output).
  - Main loop, one k-chunk (128) per step, per-qb(512) psum accumulators:
      scoresT[k, 2, q] <- one 2-bank psum supertile per chunk holding BOTH
      heads ([:,0,:] head A via kt rows 0:64, [:,1,:] head B via rows 64:128);
      the two QK matmuls are row-tile concurrent (disjoint row groups).
      exp: ONE FD=1024 op per chunk covering both heads, alternating engines
      by chunk parity to halve the per-op overhead (352cyc ScalarE/120cyc DVE):
        * kc even: ScalarE ACTIVATE exact exp (scale=1/8 folded)
        * kc odd:  VectorE tensor_scalar Schraudolph exp (i16 bitcast bf16,
          ~2% element error, zero-mean so softmax renormalization cancels)
      PV: lhsT = [V_chunk | ones] (65 cols); 2 matmuls (heads A/B) accumulate
      into [65, 512] psum over 32 chunks.
  - Epilogue per (qb, head): outT psum -> sbuf bf16 (head A copy on ScalarE,
    head B on VectorE), transpose back via 4 bf16 matmuls vs identity into
    scratch placed in the just-freed pso bank, fast reciprocal of col 64,
    broadcast multiply, DMA out on the gpsimd queue.

PSUM budget (8 banks): score supertiles [128,1024] x3 bufs = 6 banks,
PV-out A/B = 2 banks; epilogue transpose scratch reuses the pso banks.

Engine budget per chunk (~650ns pace): PE 1536cyc/2.4GHz = 640ns
(QK pair 512 + PV 2x512); ScalarE ~574ns (exp every other chunk) + epilogue
copies; VectorE ~596ns + casts + epilogue recip/mul; GpSimd: DMA issue only.
"""

import numpy as np

from concourse import bacc, mybir, tile
from concourse.bass_utils import run_bass_kernel_spmd
from concourse.masks import make_identity

P = 128          # partitions
S = 4096         # sequence length
D = 64           # head dim
NH = 4           # heads per core
NC = S // P      # 32 k-chunks of 128
QB = 512         # q block (psum bank free size in fp32)
NQ = S // QB     # 8 q blocks
NQTR = 4         # DMA quarters
CPQ = NC // NQTR # chunks per quarter (8)
SQ = S // NQTR   # seq elems per quarter (1024)
FP32 = mybir.dt.float32
BF16 = mybir.dt.bfloat16
I16 = mybir.dt.int16

N_CORES = 8
SCALE = 1.0 / np.sqrt(np.float32(D))  # 0.125

# Schraudolph exp-as-bf16-bits constants (see module docstring).
# i16 = round(EXP_A * s + EXP_B); bits -> bf16 ~= exp(s * SCALE).
EXP_A = float(128 * np.log2(np.e) * SCALE)
EXP_B = 16248.7807254998


def build():
    nc = bacc.Bacc("TRN2", target_bir_lowering=False)
    q_d = nc.dram_tensor("Q", (NH, S, D), FP32, kind="ExternalInput")
    k_d = nc.dram_tensor("K", (NH, S, D), FP32, kind="ExternalInput")
    v_d = nc.dram_tensor("V", (NH, S, D), FP32, kind="ExternalInput")
    o_d = nc.dram_tensor("out", (NH, S, D), FP32, kind="ExternalOutput")
    # DRAM bounce buffers for the bf16 transpose: rows interleave the two
    # heads of a pair ([s, 2, D] -> transpose input [s, 128]).
    qstg_d = nc.dram_tensor("qstg", (NH // 2, S, 2, D), BF16, kind="Internal")
    kstg_d = nc.dram_tensor("kstg", (NH // 2, S, 2, D), BF16, kind="Internal")

    with tile.TileContext(nc) as tc:
        with (
            tc.tile_pool(name="const", bufs=1) as const_pool,
            tc.tile_pool(name="stg", bufs=3) as stg_pool,
            tc.tile_pool(name="qt", bufs=2) as qt_pool,
            tc.tile_pool(name="kt", bufs=2) as kt_pool,
            tc.tile_pool(name="vsb", bufs=2) as vsb_pool,
            tc.tile_pool(name="pt", bufs=3) as pt_pool,
            tc.tile_pool(name="osb", bufs=4) as osb_pool,
            tc.tile_pool(name="fin", bufs=4) as fin_pool,
            tc.tile_pool(name="recip", bufs=3) as recip_pool,
            tc.tile_pool(name="sc", bufs=3, space="PSUM") as sc_pool,
            tc.tile_pool(name="pso_a", bufs=1, space="PSUM") as pso_a_pool,
            tc.tile_pool(name="pso_b", bufs=1, space="PSUM") as pso_b_pool,
        ):
            ident = const_pool.tile([P, P], BF16)
            make_identity(nc, ident)

            # preload the exp table-set (~2.7us) before any data arrives
            tl_src = const_pool.tile([P, 1], FP32)
            nc.vector.memset(tl_src, 0.0)
            tl_dst = const_pool.tile([P, 1], FP32)
            nc.scalar.activation(
                tl_dst, tl_src, mybir.ActivationFunctionType.Exp, scale=1.0
            )

            def load_pair(pair):
                """Issue all loads for a pair (quartered), all on DMA engines.

                Per (tensor, head, quarter): gpsimd cast-DMA HBM fp32 ->
                SBUF bf16 rows; sync DMA SBUF -> DRAM bf16 staging; sync
                16-bit transpose DMA DRAM [SQ, D] -> qt/kt [64, SQ] slice.
                V: gpsimd cast-DMA straight into vsb.
                """
                ha, hb = 2 * pair, 2 * pair + 1
                qt = qt_pool.tile([P, S], BF16, name=f"qt_{pair}", tag="qt")
                kt = kt_pool.tile([P, S], BF16, name=f"kt_{pair}", tag="kt")
                vsb = vsb_pool.tile(
                    [P, 2, NC, D + 1], BF16, name=f"vsb_{pair}", tag="vsb"
                )
                nc.gpsimd.memset(vsb[:, :, :, D : D + 1], 1.0)

                def load_quarter(g, x_d, xstg_d, xt, hw_eng):
                    """3-hop bf16 transpose pipeline for one quarter of Q/K."""
                    lo = g * SQ
                    for h_i, h in enumerate((ha, hb)):
                        stg = stg_pool.tile([P, CPQ, D], BF16, tag="stg")
                        nc.gpsimd.dma_start(
                            out=stg,
                            in_=x_d[h][lo : lo + SQ, :].rearrange(
                                "(c p) d -> p c d", p=P
                            ),
                        )
                        hw_eng.dma_start(
                            out=xstg_d[pair, lo : lo + SQ, h_i, :].rearrange(
                                "(c p) d -> p c d", p=P
                            ),
                            in_=stg,
                        )
                    hw_eng.dma_start(
                        out=xt[:, lo : lo + SQ],
                        in_=xstg_d[pair, lo : lo + SQ].rearrange("s h d -> s (h d)"),
                        transpose=True,
                    )

                def load_v_quarter(g):
                    clo = g * CPQ
                    for h_i, h in enumerate((ha, hb)):
                        nc.gpsimd.dma_start(
                            out=vsb[:, h_i, clo : clo + CPQ, 0:D],
                            in_=v_d[h].rearrange("(c p) d -> p c d", p=P)[
                                :, clo : clo + CPQ, :
                            ],
                        )

                # K and V quarters are consumed progressively from kc=0, Q
                # quarter g only from qb=2g -- so load K/V first, one Q
                # quarter early, the rest at the end.  Pair 0's Q pipeline
                # rides the (idle until compute starts) scalar queue to
                # parallelize the critical lead-in; later pairs have slack
                # and keep everything on sync.
                q_eng = nc.scalar if pair == 0 else nc.sync
                for g in range(NQTR):
                    load_quarter(g, k_d, kstg_d, kt, nc.sync)
                    load_v_quarter(g)
                    if g == 0:
                        load_quarter(0, q_d, qstg_d, qt, q_eng)
                for g in range(1, NQTR):
                    load_quarter(g, q_d, qstg_d, qt, q_eng)
                return {"qt": qt, "kt": kt, "vsb": vsb, "heads": (ha, hb)}

            LAG = 2  # PV trails QK by LAG chunks (software pipelining: the
            # in-order PE queue must not park on a PV whose exp isn't done)

            def main_pair(st):
                qt, kt, vsb = st["qt"], st["kt"], st["vsb"]
                ha, hb = st["heads"]
                pend = []   # (pt, kc, out_ta, out_tb, qb) awaiting PV
                epil = []   # deferred epilogue tails

                def emit_epilogue_tail(osb, h, qd):
                    """PE transpose + normalize + store (deferred so the PE
                    transposes don't clog the queue at the qb boundary)."""
                    ps4 = sc_pool.tile([P, QB // P, D + 1], FP32, tag="sc")
                    for j in range(QB // P):
                        nc.tensor.matmul(
                            ps4[:, j, :],
                            lhsT=osb[:, j * P : (j + 1) * P],
                            rhs=ident[0 : D + 1, 0 : D + 1],
                            start=True,
                            stop=True,
                        )
                    rec = recip_pool.tile([P, QB // P, 1], FP32)
                    nc.vector.reciprocal_approx_fast(rec, ps4[:, :, D : D + 1])
                    fin = fin_pool.tile([P, QB // P, D], FP32)
                    nc.vector.tensor_tensor(
                        fin,
                        ps4[:, :, 0:D],
                        rec.broadcast_to((P, QB // P, D)),
                        mybir.AluOpType.mult,
                    )
                    nc.gpsimd.dma_start(
                        out=o_d[h, qd : qd + QB, :].rearrange(
                            "(j p) d -> p j d", p=P
                        ),
                        in_=fin,
                    )

                def issue_pv(pt, kc, out_ta, out_tb, qb):
                    first = kc == 0
                    last = kc == NC - 1
                    nc.tensor.matmul(
                        out_ta,
                        lhsT=vsb[:, 0, kc, :],
                        rhs=pt[:, 0, :],
                        start=first,
                        stop=last,
                    )
                    nc.tensor.matmul(
                        out_tb,
                        lhsT=vsb[:, 1, kc, :],
                        rhs=pt[:, 1, :],
                        start=first,
                        stop=last,
                    )
                    if last:
                        # free the pso banks ASAP: copies now, rest deferred
                        qd = qb * QB
                        for h_i, (h, out_t) in enumerate(
                            ((ha, out_ta), (hb, out_tb))
                        ):
                            osb = osb_pool.tile([D + 1, QB], BF16)
                            if h_i == 0:
                                nc.scalar.copy(osb, out_t)
                            else:
                                nc.vector.tensor_copy(osb, out_t)
                            epil.append((osb, h, qd))
                    elif kc == 4 and epil:
                        while epil:
                            emit_epilogue_tail(*epil.pop(0))

                for qb in range(NQ):
                    q0 = qb * QB
                    out_ta = pso_a_pool.tile(
                        [D + 1, QB], FP32, tag="pso_a", name=f"ota_{ha}_{qb}"
                    )
                    out_tb = pso_b_pool.tile(
                        [D + 1, QB], FP32, tag="pso_b", name=f"otb_{ha}_{qb}"
                    )
                    for kc in range(NC):
                        sc = sc_pool.tile([P, 2, QB], FP32, tag="sc")
                        nc.tensor.matmul(
                            sc[:, 0, :],
                            lhsT=kt[0:64, kc * P : (kc + 1) * P],
                            rhs=qt[0:64, q0 : q0 + QB],
                            start=True,
                            stop=True,
                        )
                        nc.tensor.matmul(
                            sc[:, 1, :],
                            lhsT=kt[64:128, kc * P : (kc + 1) * P],
                            rhs=qt[64:128, q0 : q0 + QB],
                            start=True,
                            stop=True,
                        )
                        if kc % 2 == 0:
                            pt = pt_pool.tile([P, 2, QB], BF16, tag="pt")
                            nc.scalar.activation(
                                pt,
                                sc,
                                mybir.ActivationFunctionType.Exp,
                                scale=SCALE,
                            )
                        else:
                            pt_i = pt_pool.tile([P, 2, QB], I16, tag="pt")
                            nc.vector.tensor_scalar(
                                out=pt_i,
                                in0=sc,
                                scalar1=EXP_A,
                                scalar2=EXP_B,
                                op0=mybir.AluOpType.mult,
                                op1=mybir.AluOpType.add,
                            )
                            pt = pt_i.bitcast(BF16)
                        pend.append((pt, kc, out_ta, out_tb, qb))
                        if len(pend) > LAG:
                            issue_pv(*pend.pop(0))
                for args in pend:
                    issue_pv(*args)
                while epil:
                    emit_epilogue_tail(*epil.pop(0))

            st0 = load_pair(0)
            st1 = load_pair(1)
            main_pair(st0)
            main_pair(st1)

    nc.compile()
    return nc


_NC_CACHE = None


def _get_nc():
    global _NC_CACHE
    if _NC_CACHE is None:
        _NC_CACHE = build()
    return _NC_CACHE


def kernel(Q, K, V):
    Q = np.ascontiguousarray(np.asarray(Q, dtype=np.float32))
    K = np.ascontiguousarray(np.asarray(K, dtype=np.float32))
    V = np.ascontiguousarray(np.asarray(V, dtype=np.float32))
    B, H = Q.shape[0], Q.shape[1]
    qr = Q.reshape(B * H, S, D)
    kr = K.reshape(B * H, S, D)
    vr = V.reshape(B * H, S, D)
    in_maps = [
        {
            "Q": qr[i * NH : (i + 1) * NH],
            "K": kr[i * NH : (i + 1) * NH],
            "V": vr[i * NH : (i + 1) * NH],
        }
        for i in range(N_CORES)
    ]
    res = run_bass_kernel_spmd(_get_nc(), in_maps, core_ids=list(range(N_CORES)))
    out = np.concatenate([res.results[i]["out"] for i in range(N_CORES)], axis=0)
    return out.reshape(B, H, S, D)
